# revision 26
# baseline (speedup 1.0000x reference)
"""Trainium2 Bass kernel for a species-routed MoE readout layer.

Math (see problem reference): per atom x [512]:
  u = silu(emb[species]); scores = softmax(u @ Wr.T)  -> top-2 sparse gates
  out = sum_e gate_e * (W2_e @ silu(W1_e @ x + b1_e) + b2_e)
      + sum_s (W2_s @ silu(W1_s @ x + b1_s) + b2_s)          # 2 shared experts

The router depends only on species_idx (64 species), so the per-atom top-2
gates collapse to a host-computed 64x6 lookup table. Atoms are grouped by
their top-2 expert pair and each group is split evenly across the 8 cores so
the single SPMD program sees the same tile->active-expert pattern on every
core; interior tiles then only compute 2 routed + 2 shared expert MLPs.

Precision scheme (validated against an engine-exact numpy oracle on the full
input set: max-rel 9.7e-3 vs the 2e-2 gate):
  * Routed experts run entirely in fp8 e4m3 with DoubleRow matmuls (2 fp8
    MACs/cell/cycle, 256-wide contraction per instruction).  Their outputs
    are gated by ~1/6 (near-uniform softmax over tiny router logits), which
    attenuates fp8 quantization noise by the same factor.
  * Shared experts (unit gain, fp8 noise would dominate the error budget)
    stay in bf16.
  * All mm2 products carry a uniform x64 scale (routed fp8 W2 = W2*4 against
    x16-scaled gates; shared bf16 W2 pre-scaled x64) so routed and shared
    experts accumulate into the same PSUM banks; one DVE tensor_scalar
    (x 1/64, + sum_s b2_s) descales on the PSUM->SBUF copy.

Device per <=512-atom tile (tiles aligned to expert-pair segments):
  routed: z_m   = DR-matmul(W1*64 fp8, x fp8)               (2 instrs)
          h_m   = silu(z_m * 1/64 + b1)                     (ScalarE, bf16)
          hpm_m = (h_m + alpha_e) * (16*gate)  -> fp8       (DVE STT; alpha =
                  lstsq(W2_e, b2_e) folds the gated b2 into the gate mult)
          outps_c += DR-matmul(W2*4 fp8, hpm m-pair)
  shared: z_m   = matmul(W1 bf16, x bf16)                   (4 instrs)
          h_m   = silu(z_m + b1)                            (ScalarE, bf16)
          outps_c += matmul((64*W2)T[m,c] bf16, h_m)
  out_c  = outps_c * (1/64) + sum_s b2s_c                   (DVE tensor_scalar)
"""

import numpy as np
import ml_dtypes

import concourse.bass as bass
import concourse.mybir as mybir
from concourse import bacc, tile
from concourse.bass_utils import run_bass_kernel_spmd

BF16 = mybir.dt.bfloat16
FP8 = mybir.dt.float8e4
F32 = mybir.dt.float32
BF16_NP = ml_dtypes.bfloat16
FP8_NP = ml_dtypes.float8_e4m3
DR = mybir.MatmulPerfMode.DoubleRow

N_CORES = 8
N_ATOMS = 100000
IN_F = 512
HID = 512
OUT_F = 256
N_ROUTED = 6
N_SHARED = 2
N_EXP = N_ROUTED + N_SHARED
TOPK = 2
TILE_N = 512  # atoms per tile = one PSUM bank = max matmul moving dim
KC = IN_F // 128   # 4 contraction chunks for mm1
MC = HID // 128    # 4 hid chunks
OC = OUT_F // 128  # 2 out chunks

S1 = 64.0   # fp8 W1 pre-scale (silu descales via activation scale=1/S1)
SH = 16.0   # gate pre-scale for routed hpm quantization
S2R = 64.0  # uniform mm2 product scale (descale on PSUM->SBUF copy)

SPARSE = True  # compute only active routed experts per tile


def _silu(x):
    return x / (1.0 + np.exp(-x))


def _router_table(emb, W_router):
    """[64, 6] sparse top-2 gate table + per-species expert pair."""
    u = _silu(emb.astype(np.float32))
    logits = u @ W_router.astype(np.float32).T
    m = logits.max(axis=-1, keepdims=True)
    e = np.exp(logits - m)
    scores = e / e.sum(axis=-1, keepdims=True)
    order = np.argsort(-scores, axis=-1, kind="stable")
    top2 = order[:, :TOPK]
    wt = np.zeros_like(scores)
    rows = np.arange(scores.shape[0])[:, None]
    wt[rows, top2] = scores[rows, top2]
    return wt, top2


def _plan_sharding(species_idx, top2):
    """Group atoms by top-2 expert pair, split each group evenly over cores.

    Returns (idx_cores [N_CORES, NL] int64 with -1 padding, tiles) where
    tiles is a list of (n_atoms, active_routed_experts) per tile,
    identical for every core by construction.
    """
    n = species_idx.shape[0]
    if not SPARSE:
        assert n % N_CORES == 0
        nl = n // N_CORES
        idx_cores = np.arange(n, dtype=np.int64).reshape(N_CORES, nl)
        tiles = []
        for t0 in range(0, nl, TILE_N):
            tiles.append((min(TILE_N, nl - t0), tuple(range(N_ROUTED))))
        return idx_cores, tiles

    MIN_TILE = 64  # merge segments smaller than this into their neighbor

    pair_of_species = [tuple(sorted(top2[s])) for s in range(top2.shape[0])]
    pairs = sorted(set(pair_of_species))
    pair_id_of_species = np.array(
        [pairs.index(p) for p in pair_of_species], dtype=np.int64
    )
    atom_pair = pair_id_of_species[species_idx]

    seg_lens = []       # per-group per-core segment length
    seg_experts = []
    group_idx = []      # per-group atom index arrays
    for g, p in enumerate(pairs):
        idx_g = np.nonzero(atom_pair == g)[0]
        if idx_g.size == 0:
            continue
        L = -(-idx_g.size // N_CORES)  # ceil
        seg_lens.append(L)
        seg_experts.append(tuple(int(x) for x in p))
        group_idx.append(idx_g)

    # largest group first: deep pipeline while the clock warms, short tail
    order = np.argsort([-L for L in seg_lens], kind="stable")
    seg_lens = [seg_lens[i] for i in order]
    seg_experts = [seg_experts[i] for i in order]
    group_idx = [group_idx[i] for i in order]

    nl = sum(seg_lens)
    idx_cores = np.full((N_CORES, nl), -1, dtype=np.int64)
    off = 0
    for L, idx_g in zip(seg_lens, group_idx):
        for c in range(N_CORES):
            part = idx_g[c * L : (c + 1) * L]
            idx_cores[c, off : off + part.size] = part
        off += L

    # Variable-size tiles aligned to segment boundaries: each tile covers a
    # single expert pair (tiny segments merge into their neighbor).
    tiles = []
    pend_n, pend_e = 0, set()
    for L, p in zip(seg_lens, seg_experts):
        pend_n += L
        pend_e.update(p)
        if pend_n < MIN_TILE:
            continue
        k = -(-pend_n // TILE_N)
        q, r = divmod(pend_n, k)
        for i in range(k):
            tiles.append((q + (1 if i < r else 0), tuple(sorted(pend_e))))
        pend_n, pend_e = 0, set()
    if pend_n:
        if tiles:
            n0, e0 = tiles.pop()
            pend_n += n0
            pend_e.update(e0)
        k = -(-pend_n // TILE_N)
        q, r = divmod(pend_n, k)
        ee = tuple(sorted(pend_e))
        for i in range(k):
            tiles.append((q + (1 if i < r else 0), ee))
    assert sum(t[0] for t in tiles) == nl
    return idx_cores, tiles


def _build_program(nl, tiles):
    nc = bacc.Bacc("TRN2", target_bir_lowering=False, debug=False)

    x8_d = nc.declare_dram_parameter("x8", [IN_F, nl], FP8, isOutput=False)
    xb_d = nc.declare_dram_parameter("xb", [IN_F, nl], BF16, isOutput=False)
    w6_d = nc.declare_dram_parameter("w6", [N_ROUTED, nl], BF16, isOutput=False)
    # routed fp8 weights; shared bf16 weights
    w1t_d = nc.declare_dram_parameter("w1t", [N_ROUTED, IN_F, HID], FP8, isOutput=False)
    w2t_d = nc.declare_dram_parameter("w2t", [N_ROUTED, HID, OUT_F], FP8, isOutput=False)
    w1s_d = nc.declare_dram_parameter("w1s", [N_SHARED, IN_F, HID], BF16, isOutput=False)
    w2s_d = nc.declare_dram_parameter("w2s", [N_SHARED, HID, OUT_F], BF16, isOutput=False)
    # packed small constants, one DMA: [b1 (N_EXP*MC) | alpha (N_ROUTED*MC) |
    # b2s (OC)].  alpha[e] solves W2_e @ alpha_e = b2_e (host lstsq), so the
    # gated b2 rides the gate multiply: W2_e@(w*(h+alpha)) = w*(W2_e h+b2_e)
    NCST = N_EXP * MC + N_ROUTED * MC + OC
    cst_d = nc.declare_dram_parameter("cst", [128, NCST], F32, isOutput=False)
    outT_d = nc.declare_dram_parameter("outT", [OUT_F, nl], F32, isOutput=True)

    with tile.TileContext(nc) as tc:
        with (
            tc.tile_pool(name="consts", bufs=1) as consts,
            tc.tile_pool(name="x8p", bufs=4) as x8p,
            tc.tile_pool(name="xbp", bufs=4) as xbp,
            tc.tile_pool(name="w6p", bufs=2) as w6p,
            tc.tile_pool(name="wbcp", bufs=4) as wbcp,
            tc.tile_pool(name="hps", bufs=4, space="PSUM") as hpsp,
            tc.tile_pool(name="hp", bufs=10) as hp_pool,
            tc.tile_pool(name="hpm", bufs=3) as hpm_pool,
            tc.tile_pool(name="ops", bufs=4, space="PSUM") as outps_pool,
            tc.tile_pool(name="osb", bufs=3) as osb_pool,
        ):
            # ---- constants / weights preload ----
            cst_sb = consts.tile([128, NCST], F32, name="cst_sb")
            AOFF = N_EXP * MC          # alpha column offset in cst
            B2OFF = AOFF + N_ROUTED * MC  # b2s column offset in cst
            ones_sb = consts.tile([1, TILE_N], BF16, name="ones_sb")

            # Per-expert weight tiles, streamed in first-use order so tile 0
            # can start after the first experts' weights land.
            eorder = []
            for _, routed in tiles:
                for e in list(routed) + [N_ROUTED + s for s in range(N_SHARED)]:
                    if e not in eorder:
                        eorder.append(e)
                if len(eorder) == N_EXP:
                    break
            for e in range(N_EXP):
                if e not in eorder:
                    eorder.append(e)

            w1t_v = w1t_d.rearrange("e (k p) h -> e p k h", p=128)
            w2t_v = w2t_d.rearrange("e (m p) o -> e p m o", p=128)
            w1s_v = w1s_d.rearrange("e (k p) h -> e p k h", p=128)
            w2s_v = w2s_d.rearrange("e (m p) o -> e p m o", p=128)
            w1_sb = {}
            w2_sb = {}

            def load_w1(e, q=None):
                q = q or nc.sync
                if e < N_ROUTED:
                    w1_sb[e] = consts.tile([128, KC, HID], FP8, name=f"w1e{e}")
                    q.dma_start(w1_sb[e][:], w1t_v[e])
                else:
                    w1_sb[e] = consts.tile([128, KC, HID], BF16, name=f"w1e{e}")
                    q.dma_start(w1_sb[e][:], w1s_v[e - N_ROUTED])

            def load_w2(e, q=None):
                q = q or nc.sync
                if e < N_ROUTED:
                    w2_sb[e] = consts.tile([128, MC, OUT_F], FP8, name=f"w2e{e}")
                    q.dma_start(w2_sb[e][:], w2t_v[e])
                else:
                    w2_sb[e] = consts.tile([128, MC, OUT_F], BF16, name=f"w2e{e}")
                    q.dma_start(w2_sb[e][:], w2s_v[e - N_ROUTED])

            def load_expert_weights(e, q=None):
                load_w1(e, q)
                load_w2(e, q)

            # DMA issue order is the critical path at kernel start (~700ns
            # queue cost per issue): tile 0's x8 + first expert's W1 + silu
            # consts lead, the rest streams behind in first-use order.
            x8_v = x8_d.rearrange("(k p) a -> p k a", p=128)
            xb_v = xb_d.rearrange("(k p) a -> p k a", p=128)
            n0 = tiles[0][0]
            t0_x8 = x8p.tile([128, KC, TILE_N], FP8, name="x8_sb", tag="x8")
            nc.sync.dma_start(t0_x8[:, :, :n0], x8_v[:, :, 0:n0])
            load_w1(eorder[0])
            nc.sync.dma_start(cst_sb[:], cst_d[:])
            t0_w6 = w6p.tile([1, N_ROUTED, TILE_N], BF16, name="w6row", tag="w6r")
            nc.sync.dma_start(t0_w6[0:1, :, :n0], w6_d[:, 0:n0])
            load_w1(eorder[1])
            t0_xb = xbp.tile([128, KC, TILE_N], BF16, name="xb_sb", tag="xb")
            nc.sync.dma_start(t0_xb[:, :, :n0], xb_v[:, :, 0:n0])
            load_w1(eorder[2])
            load_w2(eorder[0])
            load_w2(eorder[1])

            # Warm the PE HAM clock gate (cold = 1.2 instead of 2.4 GHz) and
            # the ScalarE activation table while the first DMAs are in flight.
            nc.vector.memset(ones_sb[:], 1.0)
            warm_sb = consts.tile([128, 1], F32, name="warm_sb")
            for _ in range(8):
                warm_ps = hpsp.tile([128, TILE_N], F32, name="warm_ps", tag="hps")
                nc.tensor.matmul(
                    warm_ps[:, :], ones_sb[0:1, 0:128], ones_sb[0:1, :],
                    start=True, stop=True,
                )
            nc.scalar.activation(
                warm_sb[:, :], cst_sb[:, 0:1],
                mybir.ActivationFunctionType.Silu,
            )

            # ---- main loop over atom tiles ----
            outT_v = outT_d.rearrange("(c p) a -> p c a", p=128)
            a0 = 0
            for t, (n, routed) in enumerate(tiles):
                experts = list(routed) + [N_ROUTED + s for s in range(N_SHARED)]

                if t == 0:
                    x8_sb, xb_sb, w6row = t0_x8, t0_xb, t0_w6
                else:
                    x8_sb = x8p.tile([128, KC, TILE_N], FP8, name="x8_sb", tag="x8")
                    nc.sync.dma_start(x8_sb[:, :, :n], x8_v[:, :, a0 : a0 + n])
                    xb_sb = xbp.tile([128, KC, TILE_N], BF16, name="xb_sb", tag="xb")
                    nc.sync.dma_start(xb_sb[:, :, :n], xb_v[:, :, a0 : a0 + n])
                    # gate rows packed onto partition 0
                    w6row = w6p.tile([1, N_ROUTED, TILE_N], BF16, name="w6row", tag="w6r")
                    nc.sync.dma_start(w6row[0:1, :, :n], w6_d[:, a0 : a0 + n])

                # per-atom gates broadcast across 128 partitions (GPSIMD,
                # keeps PE free)
                wsb = {}
                for e in routed:
                    wsb_e = wbcp.tile([128, TILE_N], BF16, name="wsb", tag="wbc")
                    nc.gpsimd.partition_broadcast(
                        wsb_e[:, :n], w6row[0:1, e, :n]
                    )
                    wsb[e] = wsb_e

                if t == 0:
                    # one-time bulk weight stream on the gpsimd DMA queue so
                    # its ~8us of queue-issue time doesn't delay tiles 1-4's
                    # input prefetch on the sync queue; emitted after tile
                    # 0's broadcasts so those lead the gpsimd queue
                    load_w2(eorder[2], nc.gpsimd)
                    for e in eorder[3:]:
                        load_expert_weights(e, nc.gpsimd)

                # output accumulators (opened by the first mm2 below)
                outps = [
                    outps_pool.tile([128, TILE_N], F32, name="ops", tag="ops")
                    for _ in range(OC)
                ]
                state = {"first": True}

                def emit_mm2(e, shared, src, last_e):
                    # src: hpm tile (routed) or list of h tiles (shared)
                    if shared:
                        for m in range(MC):
                            for c in range(OC):
                                nc.tensor.matmul(
                                    outps[c][:, :n],
                                    w2_sb[e][:, m, c * 128 : (c + 1) * 128],
                                    src[m][:, :n],
                                    start=state["first"],
                                    stop=(last_e and m == MC - 1),
                                )
                            state["first"] = False
                    else:
                        for mp in range(MC // 2):
                            for c in range(OC):
                                nc.tensor.matmul(
                                    outps[c][:, :n],
                                    w2_sb[e][:, 2 * mp : 2 * mp + 2, c * 128 : (c + 1) * 128],
                                    src[:, 2 * mp : 2 * mp + 2, :n],
                                    start=state["first"],
                                    stop=(last_e and mp == MC // 2 - 1),
                                    perf_mode=DR,
                                )
                            state["first"] = False

                # software pipeline: expert i's mm2 block is emitted after
                # expert i+1's mm1 block, so the silu/STT chain feeding each
                # mm2 is covered by PE work instead of stalling the PE
                pending = None
                for ei, e in enumerate(experts):
                    last_e = ei == len(experts) - 1
                    shared = e >= N_ROUTED
                    if shared:
                        src = []
                    else:
                        src = hpm_pool.tile(
                            [128, MC, TILE_N], FP8, name="hpm_sb", tag="hpm"
                        )
                    for m in range(MC):
                        hps = hpsp.tile([128, TILE_N], F32, name="hps", tag="hps")
                        if shared:
                            for k in range(KC):
                                nc.tensor.matmul(
                                    hps[:, :n],
                                    w1_sb[e][:, k, m * 128 : (m + 1) * 128],
                                    xb_sb[:, k, :n],
                                    start=(k == 0),
                                    stop=(k == KC - 1),
                                )
                        else:
                            for kp in range(KC // 2):
                                nc.tensor.matmul(
                                    hps[:, :n],
                                    w1_sb[e][:, 2 * kp : 2 * kp + 2, m * 128 : (m + 1) * 128],
                                    x8_sb[:, 2 * kp : 2 * kp + 2, :n],
                                    start=(kp == 0),
                                    stop=(kp == KC // 2 - 1),
                                    perf_mode=DR,
                                )
                        h_sb = hp_pool.tile([128, TILE_N], BF16, name="h_sb", tag="h")
                        nc.scalar.activation(
                            h_sb[:, :n], hps[:, :n],
                            mybir.ActivationFunctionType.Silu,
                            bias=cst_sb[:, e * MC + m : e * MC + m + 1],
                            scale=(1.0 if shared else 1.0 / S1),
                        )
                        if shared:
                            src.append(h_sb)
                        else:
                            ac = e * MC + m
                            nc.vector.scalar_tensor_tensor(
                                src[:, m, :n],
                                h_sb[:, :n],
                                cst_sb[:, AOFF + ac : AOFF + ac + 1],
                                wsb[e][:, :n],
                                mybir.AluOpType.add,
                                mybir.AluOpType.mult,
                            )
                    if pending is not None:
                        emit_mm2(*pending)
                    pending = (e, shared, src, last_e)
                emit_mm2(*pending)

                # psum -> sbuf: uniform 1/S2R descale + shared b2 bias (DVE)
                osb = osb_pool.tile([128, OC, TILE_N], F32, name="osb", tag="osb")
                for c in range(OC):
                    nc.vector.tensor_scalar(
                        osb[:, c, :n], outps[c][:, :n],
                        1.0 / S2R, cst_sb[:, B2OFF + c : B2OFF + c + 1],
                        mybir.AluOpType.mult, mybir.AluOpType.add,
                    )
                nc.sync.dma_start(
                    outT_v[:, :, a0 : a0 + n], osb[:, :, :n]
                )
                a0 += n

    nc.compile()
    return nc


def _alpha_pack(rW2, rb2):
    """alpha_e = min-norm solution of W2_e @ alpha = b2_e, packed per-chunk."""
    alphas = []
    for e in range(N_ROUTED):
        a, *_ = np.linalg.lstsq(rW2[e].astype(np.float64), rb2[e].astype(np.float64))
        alphas.append(a)
    al = np.stack(alphas).astype(np.float32)  # [6, HID]
    return np.ascontiguousarray(
        al.reshape(N_ROUTED, MC, 128).transpose(2, 0, 1).reshape(128, N_ROUTED * MC)
    )


def _prep_host(inputs):
    feats = np.asarray(inputs["features"], dtype=np.float32)
    species = np.asarray(inputs["species_idx"]).astype(np.int64)
    emb = np.asarray(inputs["emb"], dtype=np.float32)
    Wr = np.asarray(inputs["W_router"], dtype=np.float32)
    rW1 = np.asarray(inputs["rW1"], dtype=np.float32)
    rb1 = np.asarray(inputs["rb1"], dtype=np.float32)
    rW2 = np.asarray(inputs["rW2"], dtype=np.float32)
    rb2 = np.asarray(inputs["rb2"], dtype=np.float32)
    sW1 = np.asarray(inputs["sW1"], dtype=np.float32)
    sb1 = np.asarray(inputs["sb1"], dtype=np.float32)
    sW2 = np.asarray(inputs["sW2"], dtype=np.float32)
    sb2 = np.asarray(inputs["sb2"], dtype=np.float32)

    wt_table, top2 = _router_table(emb, Wr)
    idx_cores, tiles = _plan_sharding(species, top2)
    nl = idx_cores.shape[1]
    w_atoms = wt_table[species] * SH  # [n, 6] f32, pre-scaled gates

    b1 = np.concatenate([rb1, sb1], axis=0)  # [8, HID]

    shared = {
        "w1t": np.ascontiguousarray(
            (rW1 * S1).transpose(0, 2, 1)
        ).astype(FP8_NP),
        "w2t": np.ascontiguousarray(
            (rW2 * (S2R / SH)).transpose(0, 2, 1)
        ).astype(FP8_NP),
        "w1s": np.ascontiguousarray(sW1.transpose(0, 2, 1)).astype(BF16_NP),
        "w2s": np.ascontiguousarray(
            (sW2 * S2R).transpose(0, 2, 1)
        ).astype(BF16_NP),
        "cst": np.ascontiguousarray(
            np.concatenate(
                [
                    b1.reshape(N_EXP, MC, 128)
                    .transpose(2, 0, 1)
                    .reshape(128, N_EXP * MC),
                    _alpha_pack(rW2, rb2),
                    sb2.sum(axis=0).reshape(OC, 128).T,
                ],
                axis=1,
            ).astype(np.float32)
        ),
    }

    in_maps = []
    for c in range(N_CORES):
        idx = idx_cores[c]
        valid = idx >= 0
        iv = idx[valid]
        fT = np.ascontiguousarray(feats[iv].T)
        x8 = np.zeros((IN_F, nl), dtype=FP8_NP)
        x8[:, valid] = fT.astype(FP8_NP)
        xb = np.zeros((IN_F, nl), dtype=BF16_NP)
        xb[:, valid] = fT.astype(BF16_NP)
        w6 = np.zeros((N_ROUTED, nl), dtype=BF16_NP)
        w6[:, valid] = np.ascontiguousarray(w_atoms[iv].T).astype(BF16_NP)
        in_maps.append({"x8": x8, "xb": xb, "w6": w6, **shared})
    return in_maps, idx_cores, tiles, nl, feats.shape[0]


_PROGRAM_CACHE = {}


def _get_program(nl, tiles):
    key = (nl, tuple(tiles))
    if key not in _PROGRAM_CACHE:
        _PROGRAM_CACHE[key] = _build_program(nl, tiles)
    return _PROGRAM_CACHE[key]


# Set TRACE=True (e.g. from a test harness) to capture a neuron-profile trace;
# the full BassKernelResults of the last run is kept in LAST_RESULTS.
TRACE = False
LAST_RESULTS = None


def kernel(**inputs):
    global LAST_RESULTS
    in_maps, idx_cores, tiles, nl, n_atoms = _prep_host(inputs)
    nc = _get_program(nl, tiles)
    res = run_bass_kernel_spmd(nc, in_maps, list(range(N_CORES)), trace=TRACE)
    LAST_RESULTS = res
    out = np.zeros((n_atoms, OUT_F), dtype=np.float32)
    for c in range(N_CORES):
        idx = idx_cores[c]
        valid = idx >= 0
        outT = res.results[c]["outT"]  # [OUT_F, nl] f32
        out[idx[valid]] = outT[:, valid].T
    return out


# revision 27
# speedup vs baseline: 1.0244x; 1.0244x over previous
"""Trainium2 Bass kernel for a species-routed MoE readout layer.

Math (see problem reference): per atom x [512]:
  u = silu(emb[species]); scores = softmax(u @ Wr.T)  -> top-2 sparse gates
  out = sum_e gate_e * (W2_e @ silu(W1_e @ x + b1_e) + b2_e)
      + sum_s (W2_s @ silu(W1_s @ x + b1_s) + b2_s)          # 2 shared experts

The router depends only on species_idx (64 species), so the per-atom top-2
gates collapse to a host-computed 64x6 lookup table. Atoms are grouped by
their top-2 expert pair and each group is split evenly across the 8 cores so
the single SPMD program sees the same tile->active-expert pattern on every
core; interior tiles then only compute 2 routed + 2 shared expert MLPs.

Precision scheme (validated against an engine-exact numpy oracle on the full
input set: max-rel 9.7e-3 vs the 2e-2 gate):
  * Routed experts run entirely in fp8 e4m3 with DoubleRow matmuls (2 fp8
    MACs/cell/cycle, 256-wide contraction per instruction).  Their outputs
    are gated by ~1/6 (near-uniform softmax over tiny router logits), which
    attenuates fp8 quantization noise by the same factor.
  * Shared experts (unit gain, fp8 noise would dominate the error budget)
    stay in bf16.
  * All mm2 products carry a uniform x64 scale (routed fp8 W2 = W2*4 against
    x16-scaled gates; shared bf16 W2 pre-scaled x64) so routed and shared
    experts accumulate into the same PSUM banks; one DVE tensor_scalar
    (x 1/64, + sum_s b2_s) descales on the PSUM->SBUF copy.

Device per <=512-atom tile (tiles aligned to expert-pair segments):
  routed: z_m   = DR-matmul(W1*64 fp8, x fp8)               (2 instrs)
          h_m   = silu(z_m * 1/64 + b1)                     (ScalarE, bf16)
          hpm_m = (h_m + alpha_e) * (16*gate)  -> fp8       (DVE STT; alpha =
                  lstsq(W2_e, b2_e) folds the gated b2 into the gate mult)
          outps_c += DR-matmul(W2*4 fp8, hpm m-pair)
  shared: z_m   = matmul(W1 bf16, x bf16)                   (4 instrs)
          h_m   = silu(z_m + b1)                            (ScalarE, bf16)
          outps_c += matmul((64*W2)T[m,c] bf16, h_m)
  out_c  = outps_c * (1/64) + sum_s b2s_c                   (DVE tensor_scalar)
"""

import numpy as np
import ml_dtypes

import concourse.bass as bass
import concourse.mybir as mybir
from concourse import bacc, tile
from concourse.bass_utils import run_bass_kernel_spmd

BF16 = mybir.dt.bfloat16
FP8 = mybir.dt.float8e4
F32 = mybir.dt.float32
BF16_NP = ml_dtypes.bfloat16
FP8_NP = ml_dtypes.float8_e4m3
DR = mybir.MatmulPerfMode.DoubleRow

N_CORES = 8
N_ATOMS = 100000
IN_F = 512
HID = 512
OUT_F = 256
N_ROUTED = 6
N_SHARED = 2
N_EXP = N_ROUTED + N_SHARED
TOPK = 2
TILE_N = 512  # atoms per tile = one PSUM bank = max matmul moving dim
KC = IN_F // 128   # 4 contraction chunks for mm1
MC = HID // 128    # 4 hid chunks
OC = OUT_F // 128  # 2 out chunks

S1 = 64.0   # fp8 W1 pre-scale (silu descales via activation scale=1/S1)
SH = 16.0   # gate pre-scale for routed hpm quantization
S2R = 64.0  # uniform mm2 product scale (descale on PSUM->SBUF copy)

SPARSE = True  # compute only active routed experts per tile


def _silu(x):
    return x / (1.0 + np.exp(-x))


def _router_table(emb, W_router):
    """[64, 6] sparse top-2 gate table + per-species expert pair."""
    u = _silu(emb.astype(np.float32))
    logits = u @ W_router.astype(np.float32).T
    m = logits.max(axis=-1, keepdims=True)
    e = np.exp(logits - m)
    scores = e / e.sum(axis=-1, keepdims=True)
    order = np.argsort(-scores, axis=-1, kind="stable")
    top2 = order[:, :TOPK]
    wt = np.zeros_like(scores)
    rows = np.arange(scores.shape[0])[:, None]
    wt[rows, top2] = scores[rows, top2]
    return wt, top2


def _plan_sharding(species_idx, top2):
    """Group atoms by top-2 expert pair, split each group evenly over cores.

    Returns (idx_cores [N_CORES, NL] int64 with -1 padding, tiles) where
    tiles is a list of (n_atoms, active_routed_experts) per tile,
    identical for every core by construction.
    """
    n = species_idx.shape[0]
    if not SPARSE:
        assert n % N_CORES == 0
        nl = n // N_CORES
        idx_cores = np.arange(n, dtype=np.int64).reshape(N_CORES, nl)
        tiles = []
        for t0 in range(0, nl, TILE_N):
            tiles.append((min(TILE_N, nl - t0), tuple(range(N_ROUTED))))
        return idx_cores, tiles

    MIN_TILE = 64  # merge segments smaller than this into their neighbor

    pair_of_species = [tuple(sorted(top2[s])) for s in range(top2.shape[0])]
    pairs = sorted(set(pair_of_species))
    pair_id_of_species = np.array(
        [pairs.index(p) for p in pair_of_species], dtype=np.int64
    )
    atom_pair = pair_id_of_species[species_idx]

    seg_lens = []       # per-group per-core segment length
    seg_experts = []
    group_idx = []      # per-group atom index arrays
    for g, p in enumerate(pairs):
        idx_g = np.nonzero(atom_pair == g)[0]
        if idx_g.size == 0:
            continue
        L = -(-idx_g.size // N_CORES)  # ceil
        seg_lens.append(L)
        seg_experts.append(tuple(int(x) for x in p))
        group_idx.append(idx_g)

    # largest group first: deep pipeline while the clock warms, short tail
    order = np.argsort([-L for L in seg_lens], kind="stable")
    seg_lens = [seg_lens[i] for i in order]
    seg_experts = [seg_experts[i] for i in order]
    group_idx = [group_idx[i] for i in order]

    nl = sum(seg_lens)
    idx_cores = np.full((N_CORES, nl), -1, dtype=np.int64)
    off = 0
    for L, idx_g in zip(seg_lens, group_idx):
        for c in range(N_CORES):
            part = idx_g[c * L : (c + 1) * L]
            idx_cores[c, off : off + part.size] = part
        off += L

    # Variable-size tiles aligned to segment boundaries: each tile covers a
    # single expert pair (tiny segments merge into their neighbor).
    tiles = []
    pend_n, pend_e = 0, set()
    for L, p in zip(seg_lens, seg_experts):
        pend_n += L
        pend_e.update(p)
        if pend_n < MIN_TILE:
            continue
        k = -(-pend_n // TILE_N)
        q, r = divmod(pend_n, k)
        for i in range(k):
            tiles.append((q + (1 if i < r else 0), tuple(sorted(pend_e))))
        pend_n, pend_e = 0, set()
    if pend_n:
        if tiles:
            n0, e0 = tiles.pop()
            pend_n += n0
            pend_e.update(e0)
        k = -(-pend_n // TILE_N)
        q, r = divmod(pend_n, k)
        ee = tuple(sorted(pend_e))
        for i in range(k):
            tiles.append((q + (1 if i < r else 0), ee))
    assert sum(t[0] for t in tiles) == nl
    return idx_cores, tiles


def _build_program(nl, tiles):
    nc = bacc.Bacc("TRN2", target_bir_lowering=False, debug=False)

    x8_d = nc.declare_dram_parameter("x8", [IN_F, nl], FP8, isOutput=False)
    xb_d = nc.declare_dram_parameter("xb", [IN_F, nl], BF16, isOutput=False)
    w6_d = nc.declare_dram_parameter("w6", [N_ROUTED, nl], BF16, isOutput=False)
    # routed fp8 weights; shared bf16 weights
    w1t_d = nc.declare_dram_parameter("w1t", [N_ROUTED, IN_F, HID], FP8, isOutput=False)
    w2t_d = nc.declare_dram_parameter("w2t", [N_ROUTED, HID, OUT_F], FP8, isOutput=False)
    w1s_d = nc.declare_dram_parameter("w1s", [N_SHARED, IN_F, HID], BF16, isOutput=False)
    w2s_d = nc.declare_dram_parameter("w2s", [N_SHARED, HID, OUT_F], BF16, isOutput=False)
    # packed small constants, one DMA: [b1 (N_EXP*MC) | alpha (N_ROUTED*MC) |
    # b2s (OC)].  alpha[e] solves W2_e @ alpha_e = b2_e (host lstsq), so the
    # gated b2 rides the gate multiply: W2_e@(w*(h+alpha)) = w*(W2_e h+b2_e)
    NCST = N_EXP * MC + N_ROUTED * MC + OC
    cst_d = nc.declare_dram_parameter("cst", [128, NCST], F32, isOutput=False)
    outT_d = nc.declare_dram_parameter("outT", [OUT_F, nl], F32, isOutput=True)

    with tile.TileContext(nc) as tc:
        with (
            tc.tile_pool(name="consts", bufs=1) as consts,
            tc.tile_pool(name="x8p", bufs=4) as x8p,
            tc.tile_pool(name="xbp", bufs=4) as xbp,
            tc.tile_pool(name="w6p", bufs=2) as w6p,
            tc.tile_pool(name="wbcp", bufs=4) as wbcp,
            tc.tile_pool(name="hps", bufs=4, space="PSUM") as hpsp,
            tc.tile_pool(name="hp", bufs=10) as hp_pool,
            tc.tile_pool(name="hpm", bufs=3) as hpm_pool,
            tc.tile_pool(name="ops", bufs=4, space="PSUM") as outps_pool,
            tc.tile_pool(name="osb", bufs=3) as osb_pool,
        ):
            # ---- constants / weights preload ----
            cst_sb = consts.tile([128, NCST], F32, name="cst_sb")
            AOFF = N_EXP * MC          # alpha column offset in cst
            B2OFF = AOFF + N_ROUTED * MC  # b2s column offset in cst
            ones_sb = consts.tile([1, TILE_N], BF16, name="ones_sb")

            # Per-expert weight tiles, streamed in first-use order so tile 0
            # can start after the first experts' weights land.
            eorder = []
            for _, routed in tiles:
                for e in list(routed) + [N_ROUTED + s for s in range(N_SHARED)]:
                    if e not in eorder:
                        eorder.append(e)
                if len(eorder) == N_EXP:
                    break
            for e in range(N_EXP):
                if e not in eorder:
                    eorder.append(e)

            w1t_v = w1t_d.rearrange("e (k p) h -> e p k h", p=128)
            w2t_v = w2t_d.rearrange("e (m p) o -> e p m o", p=128)
            w1s_v = w1s_d.rearrange("e (k p) h -> e p k h", p=128)
            w2s_v = w2s_d.rearrange("e (m p) o -> e p m o", p=128)
            w1_sb = {}
            w2_sb = {}

            def load_w1(e):
                if e < N_ROUTED:
                    w1_sb[e] = consts.tile([128, KC, HID], FP8, name=f"w1e{e}")
                    nc.sync.dma_start(w1_sb[e][:], w1t_v[e])
                else:
                    w1_sb[e] = consts.tile([128, KC, HID], BF16, name=f"w1e{e}")
                    nc.sync.dma_start(w1_sb[e][:], w1s_v[e - N_ROUTED])

            def load_w2(e):
                if e < N_ROUTED:
                    w2_sb[e] = consts.tile([128, MC, OUT_F], FP8, name=f"w2e{e}")
                    nc.sync.dma_start(w2_sb[e][:], w2t_v[e])
                else:
                    w2_sb[e] = consts.tile([128, MC, OUT_F], BF16, name=f"w2e{e}")
                    nc.sync.dma_start(w2_sb[e][:], w2s_v[e - N_ROUTED])

            def load_expert_weights(e):
                load_w1(e)
                load_w2(e)

            # DMA issue order is the critical path at kernel start (~700ns
            # queue cost per issue): tile 0's x8 + first expert's W1 + silu
            # consts lead, the rest streams behind in first-use order.
            x8_v = x8_d.rearrange("(k p) a -> p k a", p=128)
            xb_v = xb_d.rearrange("(k p) a -> p k a", p=128)
            n0 = tiles[0][0]
            t0_x8 = x8p.tile([128, KC, TILE_N], FP8, name="x8_sb", tag="x8")
            nc.sync.dma_start(t0_x8[:, :, :n0], x8_v[:, :, 0:n0])
            load_w1(eorder[0])
            nc.sync.dma_start(cst_sb[:], cst_d[:])
            t0_w6 = w6p.tile([1, N_ROUTED, TILE_N], BF16, name="w6row", tag="w6r")
            nc.sync.dma_start(t0_w6[0:1, :, :n0], w6_d[:, 0:n0])
            load_w1(eorder[1])
            t0_xb = xbp.tile([128, KC, TILE_N], BF16, name="xb_sb", tag="xb")
            nc.sync.dma_start(t0_xb[:, :, :n0], xb_v[:, :, 0:n0])
            load_w1(eorder[2])
            load_w2(eorder[0])
            load_w2(eorder[1])

            # Warm the PE HAM clock gate (cold = 1.2 instead of 2.4 GHz) and
            # the ScalarE activation table while the first DMAs are in flight.
            nc.vector.memset(ones_sb[:], 1.0)
            warm_sb = consts.tile([128, 1], F32, name="warm_sb")
            for _ in range(8):
                warm_ps = hpsp.tile([128, TILE_N], F32, name="warm_ps", tag="hps")
                nc.tensor.matmul(
                    warm_ps[:, :], ones_sb[0:1, 0:128], ones_sb[0:1, :],
                    start=True, stop=True,
                )
            nc.scalar.activation(
                warm_sb[:, :], cst_sb[:, 0:1],
                mybir.ActivationFunctionType.Silu,
            )

            # ---- main loop over atom tiles ----
            outT_v = outT_d.rearrange("(c p) a -> p c a", p=128)
            a0 = 0
            for t, (n, routed) in enumerate(tiles):
                experts = list(routed) + [N_ROUTED + s for s in range(N_SHARED)]

                if t == 0:
                    x8_sb, xb_sb, w6row = t0_x8, t0_xb, t0_w6
                    # stream the remaining experts' weights behind tile 0's
                    # inputs; tile 0's compute covers the transfer time
                    load_w2(eorder[2])
                    for e in eorder[3:]:
                        load_expert_weights(e)
                else:
                    x8_sb = x8p.tile([128, KC, TILE_N], FP8, name="x8_sb", tag="x8")
                    nc.sync.dma_start(x8_sb[:, :, :n], x8_v[:, :, a0 : a0 + n])
                    xb_sb = xbp.tile([128, KC, TILE_N], BF16, name="xb_sb", tag="xb")
                    nc.sync.dma_start(xb_sb[:, :, :n], xb_v[:, :, a0 : a0 + n])
                    # gate rows packed onto partition 0
                    w6row = w6p.tile([1, N_ROUTED, TILE_N], BF16, name="w6row", tag="w6r")
                    nc.sync.dma_start(w6row[0:1, :, :n], w6_d[:, a0 : a0 + n])

                # per-atom gates broadcast across 128 partitions (GPSIMD,
                # keeps PE free)
                wsb = {}
                for e in routed:
                    wsb_e = wbcp.tile([128, TILE_N], BF16, name="wsb", tag="wbc")
                    nc.gpsimd.partition_broadcast(
                        wsb_e[:, :n], w6row[0:1, e, :n]
                    )
                    wsb[e] = wsb_e

                # output accumulators (opened by the first mm2 below)
                outps = [
                    outps_pool.tile([128, TILE_N], F32, name="ops", tag="ops")
                    for _ in range(OC)
                ]
                state = {"first": True}

                def emit_mm2(e, shared, src, last_e):
                    # src: hpm tile (routed) or list of h tiles (shared)
                    if shared:
                        for m in range(MC):
                            for c in range(OC):
                                nc.tensor.matmul(
                                    outps[c][:, :n],
                                    w2_sb[e][:, m, c * 128 : (c + 1) * 128],
                                    src[m][:, :n],
                                    start=state["first"],
                                    stop=(last_e and m == MC - 1),
                                )
                            state["first"] = False
                    else:
                        for mp in range(MC // 2):
                            for c in range(OC):
                                nc.tensor.matmul(
                                    outps[c][:, :n],
                                    w2_sb[e][:, 2 * mp : 2 * mp + 2, c * 128 : (c + 1) * 128],
                                    src[:, 2 * mp : 2 * mp + 2, :n],
                                    start=state["first"],
                                    stop=(last_e and mp == MC // 2 - 1),
                                    perf_mode=DR,
                                )
                            state["first"] = False

                # software pipeline: expert i's mm2 block is emitted after
                # expert i+1's mm1 block, so the silu/STT chain feeding each
                # mm2 is covered by PE work instead of stalling the PE
                pending = None
                for ei, e in enumerate(experts):
                    last_e = ei == len(experts) - 1
                    shared = e >= N_ROUTED
                    if shared:
                        src = []
                    else:
                        src = hpm_pool.tile(
                            [128, MC, TILE_N], FP8, name="hpm_sb", tag="hpm"
                        )
                    for m in range(MC):
                        hps = hpsp.tile([128, TILE_N], F32, name="hps", tag="hps")
                        if shared:
                            for k in range(KC):
                                nc.tensor.matmul(
                                    hps[:, :n],
                                    w1_sb[e][:, k, m * 128 : (m + 1) * 128],
                                    xb_sb[:, k, :n],
                                    start=(k == 0),
                                    stop=(k == KC - 1),
                                )
                        else:
                            for kp in range(KC // 2):
                                nc.tensor.matmul(
                                    hps[:, :n],
                                    w1_sb[e][:, 2 * kp : 2 * kp + 2, m * 128 : (m + 1) * 128],
                                    x8_sb[:, 2 * kp : 2 * kp + 2, :n],
                                    start=(kp == 0),
                                    stop=(kp == KC // 2 - 1),
                                    perf_mode=DR,
                                )
                        h_sb = hp_pool.tile([128, TILE_N], BF16, name="h_sb", tag="h")
                        nc.scalar.activation(
                            h_sb[:, :n], hps[:, :n],
                            mybir.ActivationFunctionType.Silu,
                            bias=cst_sb[:, e * MC + m : e * MC + m + 1],
                            scale=(1.0 if shared else 1.0 / S1),
                        )
                        if shared:
                            src.append(h_sb)
                        else:
                            ac = e * MC + m
                            nc.vector.scalar_tensor_tensor(
                                src[:, m, :n],
                                h_sb[:, :n],
                                cst_sb[:, AOFF + ac : AOFF + ac + 1],
                                wsb[e][:, :n],
                                mybir.AluOpType.add,
                                mybir.AluOpType.mult,
                            )
                    if pending is not None:
                        emit_mm2(*pending)
                    pending = (e, shared, src, last_e)
                emit_mm2(*pending)

                # psum -> sbuf: uniform 1/S2R descale + shared b2 bias (DVE)
                osb = osb_pool.tile([128, OC, TILE_N], F32, name="osb", tag="osb")
                for c in range(OC):
                    nc.vector.tensor_scalar(
                        osb[:, c, :n], outps[c][:, :n],
                        1.0 / S2R, cst_sb[:, B2OFF + c : B2OFF + c + 1],
                        mybir.AluOpType.mult, mybir.AluOpType.add,
                    )
                nc.sync.dma_start(
                    outT_v[:, :, a0 : a0 + n], osb[:, :, :n]
                )
                a0 += n

    nc.compile()
    return nc


def _alpha_pack(rW2, rb2):
    """alpha_e = min-norm solution of W2_e @ alpha = b2_e, packed per-chunk."""
    alphas = []
    for e in range(N_ROUTED):
        a, *_ = np.linalg.lstsq(rW2[e].astype(np.float64), rb2[e].astype(np.float64))
        alphas.append(a)
    al = np.stack(alphas).astype(np.float32)  # [6, HID]
    return np.ascontiguousarray(
        al.reshape(N_ROUTED, MC, 128).transpose(2, 0, 1).reshape(128, N_ROUTED * MC)
    )


def _prep_host(inputs):
    feats = np.asarray(inputs["features"], dtype=np.float32)
    species = np.asarray(inputs["species_idx"]).astype(np.int64)
    emb = np.asarray(inputs["emb"], dtype=np.float32)
    Wr = np.asarray(inputs["W_router"], dtype=np.float32)
    rW1 = np.asarray(inputs["rW1"], dtype=np.float32)
    rb1 = np.asarray(inputs["rb1"], dtype=np.float32)
    rW2 = np.asarray(inputs["rW2"], dtype=np.float32)
    rb2 = np.asarray(inputs["rb2"], dtype=np.float32)
    sW1 = np.asarray(inputs["sW1"], dtype=np.float32)
    sb1 = np.asarray(inputs["sb1"], dtype=np.float32)
    sW2 = np.asarray(inputs["sW2"], dtype=np.float32)
    sb2 = np.asarray(inputs["sb2"], dtype=np.float32)

    wt_table, top2 = _router_table(emb, Wr)
    idx_cores, tiles = _plan_sharding(species, top2)
    nl = idx_cores.shape[1]
    w_atoms = wt_table[species] * SH  # [n, 6] f32, pre-scaled gates

    b1 = np.concatenate([rb1, sb1], axis=0)  # [8, HID]

    shared = {
        "w1t": np.ascontiguousarray(
            (rW1 * S1).transpose(0, 2, 1)
        ).astype(FP8_NP),
        "w2t": np.ascontiguousarray(
            (rW2 * (S2R / SH)).transpose(0, 2, 1)
        ).astype(FP8_NP),
        "w1s": np.ascontiguousarray(sW1.transpose(0, 2, 1)).astype(BF16_NP),
        "w2s": np.ascontiguousarray(
            (sW2 * S2R).transpose(0, 2, 1)
        ).astype(BF16_NP),
        "cst": np.ascontiguousarray(
            np.concatenate(
                [
                    b1.reshape(N_EXP, MC, 128)
                    .transpose(2, 0, 1)
                    .reshape(128, N_EXP * MC),
                    _alpha_pack(rW2, rb2),
                    sb2.sum(axis=0).reshape(OC, 128).T,
                ],
                axis=1,
            ).astype(np.float32)
        ),
    }

    in_maps = []
    for c in range(N_CORES):
        idx = idx_cores[c]
        valid = idx >= 0
        iv = idx[valid]
        fT = np.ascontiguousarray(feats[iv].T)
        x8 = np.zeros((IN_F, nl), dtype=FP8_NP)
        x8[:, valid] = fT.astype(FP8_NP)
        xb = np.zeros((IN_F, nl), dtype=BF16_NP)
        xb[:, valid] = fT.astype(BF16_NP)
        w6 = np.zeros((N_ROUTED, nl), dtype=BF16_NP)
        w6[:, valid] = np.ascontiguousarray(w_atoms[iv].T).astype(BF16_NP)
        in_maps.append({"x8": x8, "xb": xb, "w6": w6, **shared})
    return in_maps, idx_cores, tiles, nl, feats.shape[0]


_PROGRAM_CACHE = {}


def _get_program(nl, tiles):
    key = (nl, tuple(tiles))
    if key not in _PROGRAM_CACHE:
        _PROGRAM_CACHE[key] = _build_program(nl, tiles)
    return _PROGRAM_CACHE[key]


# Set TRACE=True (e.g. from a test harness) to capture a neuron-profile trace;
# the full BassKernelResults of the last run is kept in LAST_RESULTS.
TRACE = False
LAST_RESULTS = None


def kernel(**inputs):
    global LAST_RESULTS
    in_maps, idx_cores, tiles, nl, n_atoms = _prep_host(inputs)
    nc = _get_program(nl, tiles)
    res = run_bass_kernel_spmd(nc, in_maps, list(range(N_CORES)), trace=TRACE)
    LAST_RESULTS = res
    out = np.zeros((n_atoms, OUT_F), dtype=np.float32)
    for c in range(N_CORES):
        idx = idx_cores[c]
        valid = idx >= 0
        outT = res.results[c]["outT"]  # [OUT_F, nl] f32
        out[idx[valid]] = outT[:, valid].T
    return out


# revision 28
# speedup vs baseline: 1.0290x; 1.0044x over previous
"""Trainium2 Bass kernel for a species-routed MoE readout layer.

Math (see problem reference): per atom x [512]:
  u = silu(emb[species]); scores = softmax(u @ Wr.T)  -> top-2 sparse gates
  out = sum_e gate_e * (W2_e @ silu(W1_e @ x + b1_e) + b2_e)
      + sum_s (W2_s @ silu(W1_s @ x + b1_s) + b2_s)          # 2 shared experts

The router depends only on species_idx (64 species), so the per-atom top-2
gates collapse to a host-computed 64x6 lookup table. Atoms are grouped by
their top-2 expert pair and each group is split evenly across the 8 cores so
the single SPMD program sees the same tile->active-expert pattern on every
core; interior tiles then only compute 2 routed + 2 shared expert MLPs.

Precision scheme (validated against an engine-exact numpy oracle on the full
input set: max-rel 9.7e-3 vs the 2e-2 gate):
  * Routed experts run entirely in fp8 e4m3 with DoubleRow matmuls (2 fp8
    MACs/cell/cycle, 256-wide contraction per instruction).  Their outputs
    are gated by ~1/6 (near-uniform softmax over tiny router logits), which
    attenuates fp8 quantization noise by the same factor.
  * Shared experts (unit gain, fp8 noise would dominate the error budget)
    stay in bf16.
  * All mm2 products carry a uniform x64 scale (routed fp8 W2 = W2*4 against
    x16-scaled gates; shared bf16 W2 pre-scaled x64) so routed and shared
    experts accumulate into the same PSUM banks; one DVE tensor_scalar
    (x 1/64, + sum_s b2_s) descales on the PSUM->SBUF copy.

Device per <=512-atom tile (tiles aligned to expert-pair segments):
  routed: z_m   = DR-matmul(W1*64 fp8, x fp8)               (2 instrs)
          h_m   = silu(z_m * 1/64 + b1)                     (ScalarE, bf16)
          hpm_m = (h_m + alpha_e) * (16*gate)  -> fp8       (DVE STT; alpha =
                  lstsq(W2_e, b2_e) folds the gated b2 into the gate mult)
          outps_c += DR-matmul(W2*4 fp8, hpm m-pair)
  shared: z_m   = matmul(W1 bf16, x bf16)                   (4 instrs)
          h_m   = silu(z_m + b1)                            (ScalarE, bf16)
          outps_c += matmul((64*W2)T[m,c] bf16, h_m)
  out_c  = outps_c * (1/64) + sum_s b2s_c                   (DVE tensor_scalar)
"""

import numpy as np
import ml_dtypes

import concourse.bass as bass
import concourse.mybir as mybir
from concourse import bacc, tile
from concourse.bass_utils import run_bass_kernel_spmd

BF16 = mybir.dt.bfloat16
FP8 = mybir.dt.float8e4
F32 = mybir.dt.float32
BF16_NP = ml_dtypes.bfloat16
FP8_NP = ml_dtypes.float8_e4m3
DR = mybir.MatmulPerfMode.DoubleRow

N_CORES = 8
N_ATOMS = 100000
IN_F = 512
HID = 512
OUT_F = 256
N_ROUTED = 6
N_SHARED = 2
N_EXP = N_ROUTED + N_SHARED
TOPK = 2
TILE_N = 512  # atoms per tile = one PSUM bank = max matmul moving dim
KC = IN_F // 128   # 4 contraction chunks for mm1
MC = HID // 128    # 4 hid chunks
OC = OUT_F // 128  # 2 out chunks

S1 = 64.0   # fp8 W1 pre-scale (silu descales via activation scale=1/S1)
SH = 16.0   # gate pre-scale for routed hpm quantization
S2R = 64.0  # uniform mm2 product scale (descale on PSUM->SBUF copy)

SPARSE = True  # compute only active routed experts per tile


def _silu(x):
    return x / (1.0 + np.exp(-x))


def _router_table(emb, W_router):
    """[64, 6] sparse top-2 gate table + per-species expert pair."""
    u = _silu(emb.astype(np.float32))
    logits = u @ W_router.astype(np.float32).T
    m = logits.max(axis=-1, keepdims=True)
    e = np.exp(logits - m)
    scores = e / e.sum(axis=-1, keepdims=True)
    order = np.argsort(-scores, axis=-1, kind="stable")
    top2 = order[:, :TOPK]
    wt = np.zeros_like(scores)
    rows = np.arange(scores.shape[0])[:, None]
    wt[rows, top2] = scores[rows, top2]
    return wt, top2


def _plan_sharding(species_idx, top2):
    """Group atoms by top-2 expert pair, split each group evenly over cores.

    Returns (idx_cores [N_CORES, NL] int64 with -1 padding, tiles) where
    tiles is a list of (n_atoms, active_routed_experts) per tile,
    identical for every core by construction.
    """
    n = species_idx.shape[0]
    if not SPARSE:
        assert n % N_CORES == 0
        nl = n // N_CORES
        idx_cores = np.arange(n, dtype=np.int64).reshape(N_CORES, nl)
        tiles = []
        for t0 in range(0, nl, TILE_N):
            tiles.append((min(TILE_N, nl - t0), tuple(range(N_ROUTED))))
        return idx_cores, tiles

    MIN_TILE = 64  # merge segments smaller than this into their neighbor

    pair_of_species = [tuple(sorted(top2[s])) for s in range(top2.shape[0])]
    pairs = sorted(set(pair_of_species))
    pair_id_of_species = np.array(
        [pairs.index(p) for p in pair_of_species], dtype=np.int64
    )
    atom_pair = pair_id_of_species[species_idx]

    seg_lens = []       # per-group per-core segment length
    seg_experts = []
    group_idx = []      # per-group atom index arrays
    for g, p in enumerate(pairs):
        idx_g = np.nonzero(atom_pair == g)[0]
        if idx_g.size == 0:
            continue
        L = -(-idx_g.size // N_CORES)  # ceil
        seg_lens.append(L)
        seg_experts.append(tuple(int(x) for x in p))
        group_idx.append(idx_g)

    # largest group first: deep pipeline while the clock warms, short tail
    order = np.argsort([-L for L in seg_lens], kind="stable")
    seg_lens = [seg_lens[i] for i in order]
    seg_experts = [seg_experts[i] for i in order]
    group_idx = [group_idx[i] for i in order]

    nl = sum(seg_lens)
    idx_cores = np.full((N_CORES, nl), -1, dtype=np.int64)
    off = 0
    for L, idx_g in zip(seg_lens, group_idx):
        for c in range(N_CORES):
            part = idx_g[c * L : (c + 1) * L]
            idx_cores[c, off : off + part.size] = part
        off += L

    # Variable-size tiles aligned to segment boundaries: each tile covers a
    # single expert pair (tiny segments merge into their neighbor).
    tiles = []
    pend_n, pend_e = 0, set()
    for L, p in zip(seg_lens, seg_experts):
        pend_n += L
        pend_e.update(p)
        if pend_n < MIN_TILE:
            continue
        k = -(-pend_n // TILE_N)
        q, r = divmod(pend_n, k)
        for i in range(k):
            tiles.append((q + (1 if i < r else 0), tuple(sorted(pend_e))))
        pend_n, pend_e = 0, set()
    if pend_n:
        if tiles:
            n0, e0 = tiles.pop()
            pend_n += n0
            pend_e.update(e0)
        k = -(-pend_n // TILE_N)
        q, r = divmod(pend_n, k)
        ee = tuple(sorted(pend_e))
        for i in range(k):
            tiles.append((q + (1 if i < r else 0), ee))
    assert sum(t[0] for t in tiles) == nl
    return idx_cores, tiles


def _build_program(nl, tiles):
    nc = bacc.Bacc("TRN2", target_bir_lowering=False, debug=False)

    x8_d = nc.declare_dram_parameter("x8", [IN_F, nl], FP8, isOutput=False)
    xb_d = nc.declare_dram_parameter("xb", [IN_F, nl], BF16, isOutput=False)
    w6_d = nc.declare_dram_parameter("w6", [N_ROUTED, nl], BF16, isOutput=False)
    # routed fp8 weights; shared bf16 weights
    w1t_d = nc.declare_dram_parameter("w1t", [N_ROUTED, IN_F, HID], FP8, isOutput=False)
    w2t_d = nc.declare_dram_parameter("w2t", [N_ROUTED, HID, OUT_F], FP8, isOutput=False)
    w1s_d = nc.declare_dram_parameter("w1s", [N_SHARED, IN_F, HID], BF16, isOutput=False)
    w2s_d = nc.declare_dram_parameter("w2s", [N_SHARED, HID, OUT_F], BF16, isOutput=False)
    # packed small constants, one DMA: [b1 (N_EXP*MC) | alpha (N_ROUTED*MC) |
    # b2s (OC)].  alpha[e] solves W2_e @ alpha_e = b2_e (host lstsq), so the
    # gated b2 rides the gate multiply: W2_e@(w*(h+alpha)) = w*(W2_e h+b2_e)
    NCST = N_EXP * MC + N_ROUTED * MC + OC
    cst_d = nc.declare_dram_parameter("cst", [128, NCST], F32, isOutput=False)
    outT_d = nc.declare_dram_parameter("outT", [OUT_F, nl], F32, isOutput=True)

    with tile.TileContext(nc) as tc:
        with (
            tc.tile_pool(name="consts", bufs=1) as consts,
            tc.tile_pool(name="x8p", bufs=4) as x8p,
            tc.tile_pool(name="xbp", bufs=4) as xbp,
            tc.tile_pool(name="w6p", bufs=2) as w6p,
            tc.tile_pool(name="wbcp", bufs=4) as wbcp,
            tc.tile_pool(name="hps", bufs=4, space="PSUM") as hpsp,
            tc.tile_pool(name="hp", bufs=10) as hp_pool,
            tc.tile_pool(name="hpm", bufs=3) as hpm_pool,
            tc.tile_pool(name="ops", bufs=4, space="PSUM") as outps_pool,
            tc.tile_pool(name="osb", bufs=3) as osb_pool,
        ):
            # ---- constants / weights preload ----
            cst_sb = consts.tile([128, NCST], F32, name="cst_sb")
            AOFF = N_EXP * MC          # alpha column offset in cst
            B2OFF = AOFF + N_ROUTED * MC  # b2s column offset in cst
            ones_sb = consts.tile([1, TILE_N], BF16, name="ones_sb")

            # Per-expert weight tiles, streamed in first-use order so tile 0
            # can start after the first experts' weights land.
            eorder = []
            for _, routed in tiles:
                for e in list(routed) + [N_ROUTED + s for s in range(N_SHARED)]:
                    if e not in eorder:
                        eorder.append(e)
                if len(eorder) == N_EXP:
                    break
            for e in range(N_EXP):
                if e not in eorder:
                    eorder.append(e)

            w1t_v = w1t_d.rearrange("e (k p) h -> e p k h", p=128)
            w2t_v = w2t_d.rearrange("e (m p) o -> e p m o", p=128)
            w1s_v = w1s_d.rearrange("e (k p) h -> e p k h", p=128)
            w2s_v = w2s_d.rearrange("e (m p) o -> e p m o", p=128)
            w1_sb = {}
            w2_sb = {}

            def load_w1(e):
                if e < N_ROUTED:
                    w1_sb[e] = consts.tile([128, KC, HID], FP8, name=f"w1e{e}")
                    nc.sync.dma_start(w1_sb[e][:], w1t_v[e])
                else:
                    w1_sb[e] = consts.tile([128, KC, HID], BF16, name=f"w1e{e}")
                    nc.sync.dma_start(w1_sb[e][:], w1s_v[e - N_ROUTED])

            def load_w2(e):
                if e < N_ROUTED:
                    w2_sb[e] = consts.tile([128, MC, OUT_F], FP8, name=f"w2e{e}")
                    nc.sync.dma_start(w2_sb[e][:], w2t_v[e])
                else:
                    w2_sb[e] = consts.tile([128, MC, OUT_F], BF16, name=f"w2e{e}")
                    nc.sync.dma_start(w2_sb[e][:], w2s_v[e - N_ROUTED])

            def load_expert_weights(e):
                load_w1(e)
                load_w2(e)

            # DMA issue order is the critical path at kernel start (~700ns
            # queue cost per issue): tile 0's x8 + first expert's W1 + silu
            # consts lead, the rest streams behind in first-use order.
            x8_v = x8_d.rearrange("(k p) a -> p k a", p=128)
            xb_v = xb_d.rearrange("(k p) a -> p k a", p=128)
            n0 = tiles[0][0]
            t0_x8 = x8p.tile([128, KC, TILE_N], FP8, name="x8_sb", tag="x8")
            nc.sync.dma_start(t0_x8[:, :, :n0], x8_v[:, :, 0:n0])
            load_w1(eorder[0])
            nc.sync.dma_start(cst_sb[:], cst_d[:])
            t0_w6 = w6p.tile([1, N_ROUTED, TILE_N], BF16, name="w6row", tag="w6r")
            nc.sync.dma_start(t0_w6[0:1, :, :n0], w6_d[:, 0:n0])
            load_w1(eorder[1])
            t0_xb = xbp.tile([128, KC, TILE_N], BF16, name="xb_sb", tag="xb")
            nc.sync.dma_start(t0_xb[:, :, :n0], xb_v[:, :, 0:n0])
            load_w1(eorder[2])
            load_w2(eorder[0])
            load_w2(eorder[1])

            # Warm the PE HAM clock gate (cold = 1.2 instead of 2.4 GHz) and
            # the ScalarE activation table while the first DMAs are in flight.
            nc.vector.memset(ones_sb[:], 1.0)
            warm_sb = consts.tile([128, 1], F32, name="warm_sb")
            for _ in range(8):
                warm_ps = hpsp.tile([128, TILE_N], F32, name="warm_ps", tag="hps")
                nc.tensor.matmul(
                    warm_ps[:, :], ones_sb[0:1, 0:128], ones_sb[0:1, :],
                    start=True, stop=True,
                )
            nc.scalar.activation(
                warm_sb[:, :], cst_sb[:, 0:1],
                mybir.ActivationFunctionType.Silu,
            )

            # ---- main loop over atom tiles ----
            outT_v = outT_d.rearrange("(c p) a -> p c a", p=128)
            a0 = 0
            for t, (n, routed) in enumerate(tiles):
                experts = list(routed) + [N_ROUTED + s for s in range(N_SHARED)]

                if t == 0:
                    x8_sb, xb_sb, w6row = t0_x8, t0_xb, t0_w6
                    # tile 0 still needs the shared experts' remaining
                    # weights; the cold experts (first used at tile ~7+)
                    # spread over tiles 1..4 below so their ~700ns/issue
                    # queue cost doesn't delay tiles 1-4's input prefetch
                    load_w2(eorder[2])
                    load_expert_weights(eorder[3])
                else:
                    x8_sb = x8p.tile([128, KC, TILE_N], FP8, name="x8_sb", tag="x8")
                    nc.sync.dma_start(x8_sb[:, :, :n], x8_v[:, :, a0 : a0 + n])
                    xb_sb = xbp.tile([128, KC, TILE_N], BF16, name="xb_sb", tag="xb")
                    nc.sync.dma_start(xb_sb[:, :, :n], xb_v[:, :, a0 : a0 + n])
                    # gate rows packed onto partition 0
                    w6row = w6p.tile([1, N_ROUTED, TILE_N], BF16, name="w6row", tag="w6r")
                    nc.sync.dma_start(w6row[0:1, :, :n], w6_d[:, a0 : a0 + n])
                    if t - 1 < len(eorder[4:]):
                        load_expert_weights(eorder[4 + t - 1])

                # per-atom gates broadcast across 128 partitions (GPSIMD,
                # keeps PE free)
                wsb = {}
                for e in routed:
                    wsb_e = wbcp.tile([128, TILE_N], BF16, name="wsb", tag="wbc")
                    nc.gpsimd.partition_broadcast(
                        wsb_e[:, :n], w6row[0:1, e, :n]
                    )
                    wsb[e] = wsb_e

                # output accumulators (opened by the first mm2 below)
                outps = [
                    outps_pool.tile([128, TILE_N], F32, name="ops", tag="ops")
                    for _ in range(OC)
                ]
                state = {"first": True}

                def emit_mm2(e, shared, src, last_e):
                    # src: hpm tile (routed) or list of h tiles (shared)
                    if shared:
                        for m in range(MC):
                            for c in range(OC):
                                nc.tensor.matmul(
                                    outps[c][:, :n],
                                    w2_sb[e][:, m, c * 128 : (c + 1) * 128],
                                    src[m][:, :n],
                                    start=state["first"],
                                    stop=(last_e and m == MC - 1),
                                )
                            state["first"] = False
                    else:
                        for mp in range(MC // 2):
                            for c in range(OC):
                                nc.tensor.matmul(
                                    outps[c][:, :n],
                                    w2_sb[e][:, 2 * mp : 2 * mp + 2, c * 128 : (c + 1) * 128],
                                    src[:, 2 * mp : 2 * mp + 2, :n],
                                    start=state["first"],
                                    stop=(last_e and mp == MC // 2 - 1),
                                    perf_mode=DR,
                                )
                            state["first"] = False

                # software pipeline: expert i's mm2 block is emitted after
                # expert i+1's mm1 block, so the silu/STT chain feeding each
                # mm2 is covered by PE work instead of stalling the PE
                pending = None
                for ei, e in enumerate(experts):
                    last_e = ei == len(experts) - 1
                    shared = e >= N_ROUTED
                    if shared:
                        src = []
                    else:
                        src = hpm_pool.tile(
                            [128, MC, TILE_N], FP8, name="hpm_sb", tag="hpm"
                        )
                    for m in range(MC):
                        hps = hpsp.tile([128, TILE_N], F32, name="hps", tag="hps")
                        if shared:
                            for k in range(KC):
                                nc.tensor.matmul(
                                    hps[:, :n],
                                    w1_sb[e][:, k, m * 128 : (m + 1) * 128],
                                    xb_sb[:, k, :n],
                                    start=(k == 0),
                                    stop=(k == KC - 1),
                                )
                        else:
                            for kp in range(KC // 2):
                                nc.tensor.matmul(
                                    hps[:, :n],
                                    w1_sb[e][:, 2 * kp : 2 * kp + 2, m * 128 : (m + 1) * 128],
                                    x8_sb[:, 2 * kp : 2 * kp + 2, :n],
                                    start=(kp == 0),
                                    stop=(kp == KC // 2 - 1),
                                    perf_mode=DR,
                                )
                        h_sb = hp_pool.tile([128, TILE_N], BF16, name="h_sb", tag="h")
                        nc.scalar.activation(
                            h_sb[:, :n], hps[:, :n],
                            mybir.ActivationFunctionType.Silu,
                            bias=cst_sb[:, e * MC + m : e * MC + m + 1],
                            scale=(1.0 if shared else 1.0 / S1),
                        )
                        if shared:
                            src.append(h_sb)
                        else:
                            ac = e * MC + m
                            nc.vector.scalar_tensor_tensor(
                                src[:, m, :n],
                                h_sb[:, :n],
                                cst_sb[:, AOFF + ac : AOFF + ac + 1],
                                wsb[e][:, :n],
                                mybir.AluOpType.add,
                                mybir.AluOpType.mult,
                            )
                    if pending is not None:
                        emit_mm2(*pending)
                    pending = (e, shared, src, last_e)
                emit_mm2(*pending)

                # psum -> sbuf: uniform 1/S2R descale + shared b2 bias (DVE)
                osb = osb_pool.tile([128, OC, TILE_N], F32, name="osb", tag="osb")
                for c in range(OC):
                    nc.vector.tensor_scalar(
                        osb[:, c, :n], outps[c][:, :n],
                        1.0 / S2R, cst_sb[:, B2OFF + c : B2OFF + c + 1],
                        mybir.AluOpType.mult, mybir.AluOpType.add,
                    )
                nc.sync.dma_start(
                    outT_v[:, :, a0 : a0 + n], osb[:, :, :n]
                )
                a0 += n

    nc.compile()
    return nc


def _alpha_pack(rW2, rb2):
    """alpha_e = min-norm solution of W2_e @ alpha = b2_e, packed per-chunk."""
    alphas = []
    for e in range(N_ROUTED):
        a, *_ = np.linalg.lstsq(rW2[e].astype(np.float64), rb2[e].astype(np.float64))
        alphas.append(a)
    al = np.stack(alphas).astype(np.float32)  # [6, HID]
    return np.ascontiguousarray(
        al.reshape(N_ROUTED, MC, 128).transpose(2, 0, 1).reshape(128, N_ROUTED * MC)
    )


def _prep_host(inputs):
    feats = np.asarray(inputs["features"], dtype=np.float32)
    species = np.asarray(inputs["species_idx"]).astype(np.int64)
    emb = np.asarray(inputs["emb"], dtype=np.float32)
    Wr = np.asarray(inputs["W_router"], dtype=np.float32)
    rW1 = np.asarray(inputs["rW1"], dtype=np.float32)
    rb1 = np.asarray(inputs["rb1"], dtype=np.float32)
    rW2 = np.asarray(inputs["rW2"], dtype=np.float32)
    rb2 = np.asarray(inputs["rb2"], dtype=np.float32)
    sW1 = np.asarray(inputs["sW1"], dtype=np.float32)
    sb1 = np.asarray(inputs["sb1"], dtype=np.float32)
    sW2 = np.asarray(inputs["sW2"], dtype=np.float32)
    sb2 = np.asarray(inputs["sb2"], dtype=np.float32)

    wt_table, top2 = _router_table(emb, Wr)
    idx_cores, tiles = _plan_sharding(species, top2)
    nl = idx_cores.shape[1]
    w_atoms = wt_table[species] * SH  # [n, 6] f32, pre-scaled gates

    b1 = np.concatenate([rb1, sb1], axis=0)  # [8, HID]

    shared = {
        "w1t": np.ascontiguousarray(
            (rW1 * S1).transpose(0, 2, 1)
        ).astype(FP8_NP),
        "w2t": np.ascontiguousarray(
            (rW2 * (S2R / SH)).transpose(0, 2, 1)
        ).astype(FP8_NP),
        "w1s": np.ascontiguousarray(sW1.transpose(0, 2, 1)).astype(BF16_NP),
        "w2s": np.ascontiguousarray(
            (sW2 * S2R).transpose(0, 2, 1)
        ).astype(BF16_NP),
        "cst": np.ascontiguousarray(
            np.concatenate(
                [
                    b1.reshape(N_EXP, MC, 128)
                    .transpose(2, 0, 1)
                    .reshape(128, N_EXP * MC),
                    _alpha_pack(rW2, rb2),
                    sb2.sum(axis=0).reshape(OC, 128).T,
                ],
                axis=1,
            ).astype(np.float32)
        ),
    }

    in_maps = []
    for c in range(N_CORES):
        idx = idx_cores[c]
        valid = idx >= 0
        iv = idx[valid]
        fT = np.ascontiguousarray(feats[iv].T)
        x8 = np.zeros((IN_F, nl), dtype=FP8_NP)
        x8[:, valid] = fT.astype(FP8_NP)
        xb = np.zeros((IN_F, nl), dtype=BF16_NP)
        xb[:, valid] = fT.astype(BF16_NP)
        w6 = np.zeros((N_ROUTED, nl), dtype=BF16_NP)
        w6[:, valid] = np.ascontiguousarray(w_atoms[iv].T).astype(BF16_NP)
        in_maps.append({"x8": x8, "xb": xb, "w6": w6, **shared})
    return in_maps, idx_cores, tiles, nl, feats.shape[0]


_PROGRAM_CACHE = {}


def _get_program(nl, tiles):
    key = (nl, tuple(tiles))
    if key not in _PROGRAM_CACHE:
        _PROGRAM_CACHE[key] = _build_program(nl, tiles)
    return _PROGRAM_CACHE[key]


# Set TRACE=True (e.g. from a test harness) to capture a neuron-profile trace;
# the full BassKernelResults of the last run is kept in LAST_RESULTS.
TRACE = False
LAST_RESULTS = None


def kernel(**inputs):
    global LAST_RESULTS
    in_maps, idx_cores, tiles, nl, n_atoms = _prep_host(inputs)
    nc = _get_program(nl, tiles)
    res = run_bass_kernel_spmd(nc, in_maps, list(range(N_CORES)), trace=TRACE)
    LAST_RESULTS = res
    out = np.zeros((n_atoms, OUT_F), dtype=np.float32)
    for c in range(N_CORES):
        idx = idx_cores[c]
        valid = idx >= 0
        outT = res.results[c]["outT"]  # [OUT_F, nl] f32
        out[idx[valid]] = outT[:, valid].T
    return out


# revision 29
# speedup vs baseline: 1.0299x; 1.0009x over previous
"""Trainium2 Bass kernel for a species-routed MoE readout layer.

Math (see problem reference): per atom x [512]:
  u = silu(emb[species]); scores = softmax(u @ Wr.T)  -> top-2 sparse gates
  out = sum_e gate_e * (W2_e @ silu(W1_e @ x + b1_e) + b2_e)
      + sum_s (W2_s @ silu(W1_s @ x + b1_s) + b2_s)          # 2 shared experts

The router depends only on species_idx (64 species), so the per-atom top-2
gates collapse to a host-computed 64x6 lookup table. Atoms are grouped by
their top-2 expert pair and each group is split evenly across the 8 cores so
the single SPMD program sees the same tile->active-expert pattern on every
core; interior tiles then only compute 2 routed + 2 shared expert MLPs.

Precision scheme (validated against an engine-exact numpy oracle on the full
input set: max-rel 9.7e-3 vs the 2e-2 gate):
  * Routed experts run entirely in fp8 e4m3 with DoubleRow matmuls (2 fp8
    MACs/cell/cycle, 256-wide contraction per instruction).  Their outputs
    are gated by ~1/6 (near-uniform softmax over tiny router logits), which
    attenuates fp8 quantization noise by the same factor.
  * Shared experts (unit gain, fp8 noise would dominate the error budget)
    stay in bf16.
  * All mm2 products carry a uniform x64 scale (routed fp8 W2 = W2*4 against
    x16-scaled gates; shared bf16 W2 pre-scaled x64) so routed and shared
    experts accumulate into the same PSUM banks; one DVE tensor_scalar
    (x 1/64, + sum_s b2_s) descales on the PSUM->SBUF copy.

Device per <=512-atom tile (tiles aligned to expert-pair segments):
  routed: z_m   = DR-matmul(W1*64 fp8, x fp8)               (2 instrs)
          h_m   = silu(z_m * 1/64 + b1)                     (ScalarE, bf16)
          hpm_m = (h_m + alpha_e) * (16*gate)  -> fp8       (DVE STT; alpha =
                  lstsq(W2_e, b2_e) folds the gated b2 into the gate mult)
          outps_c += DR-matmul(W2*4 fp8, hpm m-pair)
  shared: z_m   = matmul(W1 bf16, x bf16)                   (4 instrs)
          h_m   = silu(z_m + b1)                            (ScalarE, bf16)
          outps_c += matmul((64*W2)T[m,c] bf16, h_m)
  out_c  = outps_c * (1/64) + sum_s b2s_c                   (DVE tensor_scalar)
"""

import numpy as np
import ml_dtypes

import concourse.bass as bass
import concourse.mybir as mybir
from concourse import bacc, tile
from concourse.bass_utils import run_bass_kernel_spmd

BF16 = mybir.dt.bfloat16
FP8 = mybir.dt.float8e4
F32 = mybir.dt.float32
BF16_NP = ml_dtypes.bfloat16
FP8_NP = ml_dtypes.float8_e4m3
DR = mybir.MatmulPerfMode.DoubleRow

N_CORES = 8
N_ATOMS = 100000
IN_F = 512
HID = 512
OUT_F = 256
N_ROUTED = 6
N_SHARED = 2
N_EXP = N_ROUTED + N_SHARED
TOPK = 2
TILE_N = 512  # atoms per tile = one PSUM bank = max matmul moving dim
KC = IN_F // 128   # 4 contraction chunks for mm1
MC = HID // 128    # 4 hid chunks
OC = OUT_F // 128  # 2 out chunks

S1 = 64.0   # fp8 W1 pre-scale (silu descales via activation scale=1/S1)
SH = 16.0   # gate pre-scale for routed hpm quantization
S2R = 64.0  # uniform mm2 product scale (descale on PSUM->SBUF copy)

SPARSE = True  # compute only active routed experts per tile


def _silu(x):
    return x / (1.0 + np.exp(-x))


def _router_table(emb, W_router):
    """[64, 6] sparse top-2 gate table + per-species expert pair."""
    u = _silu(emb.astype(np.float32))
    logits = u @ W_router.astype(np.float32).T
    m = logits.max(axis=-1, keepdims=True)
    e = np.exp(logits - m)
    scores = e / e.sum(axis=-1, keepdims=True)
    order = np.argsort(-scores, axis=-1, kind="stable")
    top2 = order[:, :TOPK]
    wt = np.zeros_like(scores)
    rows = np.arange(scores.shape[0])[:, None]
    wt[rows, top2] = scores[rows, top2]
    return wt, top2


def _plan_sharding(species_idx, top2):
    """Group atoms by top-2 expert pair, split each group evenly over cores.

    Returns (idx_cores [N_CORES, NL] int64 with -1 padding, tiles) where
    tiles is a list of (n_atoms, active_routed_experts) per tile,
    identical for every core by construction.
    """
    n = species_idx.shape[0]
    if not SPARSE:
        assert n % N_CORES == 0
        nl = n // N_CORES
        idx_cores = np.arange(n, dtype=np.int64).reshape(N_CORES, nl)
        tiles = []
        for t0 in range(0, nl, TILE_N):
            tiles.append((min(TILE_N, nl - t0), tuple(range(N_ROUTED))))
        return idx_cores, tiles

    MIN_TILE = 64  # merge segments smaller than this into their neighbor

    pair_of_species = [tuple(sorted(top2[s])) for s in range(top2.shape[0])]
    pairs = sorted(set(pair_of_species))
    pair_id_of_species = np.array(
        [pairs.index(p) for p in pair_of_species], dtype=np.int64
    )
    atom_pair = pair_id_of_species[species_idx]

    seg_lens = []       # per-group per-core segment length
    seg_experts = []
    group_idx = []      # per-group atom index arrays
    for g, p in enumerate(pairs):
        idx_g = np.nonzero(atom_pair == g)[0]
        if idx_g.size == 0:
            continue
        L = -(-idx_g.size // N_CORES)  # ceil
        seg_lens.append(L)
        seg_experts.append(tuple(int(x) for x in p))
        group_idx.append(idx_g)

    # largest group first: deep pipeline while the clock warms, short tail
    order = np.argsort([-L for L in seg_lens], kind="stable")
    seg_lens = [seg_lens[i] for i in order]
    seg_experts = [seg_experts[i] for i in order]
    group_idx = [group_idx[i] for i in order]

    nl = sum(seg_lens)
    idx_cores = np.full((N_CORES, nl), -1, dtype=np.int64)
    off = 0
    for L, idx_g in zip(seg_lens, group_idx):
        for c in range(N_CORES):
            part = idx_g[c * L : (c + 1) * L]
            idx_cores[c, off : off + part.size] = part
        off += L

    # Variable-size tiles aligned to segment boundaries: each tile covers a
    # single expert pair (tiny segments merge into their neighbor).
    tiles = []
    pend_n, pend_e = 0, set()
    for L, p in zip(seg_lens, seg_experts):
        pend_n += L
        pend_e.update(p)
        if pend_n < MIN_TILE:
            continue
        k = -(-pend_n // TILE_N)
        q, r = divmod(pend_n, k)
        for i in range(k):
            tiles.append((q + (1 if i < r else 0), tuple(sorted(pend_e))))
        pend_n, pend_e = 0, set()
    if pend_n:
        if tiles:
            n0, e0 = tiles.pop()
            pend_n += n0
            pend_e.update(e0)
        k = -(-pend_n // TILE_N)
        q, r = divmod(pend_n, k)
        ee = tuple(sorted(pend_e))
        for i in range(k):
            tiles.append((q + (1 if i < r else 0), ee))
    assert sum(t[0] for t in tiles) == nl
    return idx_cores, tiles


def _build_program(nl, tiles):
    nc = bacc.Bacc("TRN2", target_bir_lowering=False, debug=False)

    x8_d = nc.declare_dram_parameter("x8", [IN_F, nl], FP8, isOutput=False)
    xb_d = nc.declare_dram_parameter("xb", [IN_F, nl], BF16, isOutput=False)
    w6_d = nc.declare_dram_parameter("w6", [N_ROUTED, nl], BF16, isOutput=False)
    # routed fp8 weights; shared bf16 weights
    w1t_d = nc.declare_dram_parameter("w1t", [N_ROUTED, IN_F, HID], FP8, isOutput=False)
    w2t_d = nc.declare_dram_parameter("w2t", [N_ROUTED, HID, OUT_F], FP8, isOutput=False)
    w1s_d = nc.declare_dram_parameter("w1s", [N_SHARED, IN_F, HID], BF16, isOutput=False)
    w2s_d = nc.declare_dram_parameter("w2s", [N_SHARED, HID, OUT_F], BF16, isOutput=False)
    # packed small constants, one DMA: [b1 (N_EXP*MC) | alpha (N_ROUTED*MC) |
    # b2s (OC)].  alpha[e] solves W2_e @ alpha_e = b2_e (host lstsq), so the
    # gated b2 rides the gate multiply: W2_e@(w*(h+alpha)) = w*(W2_e h+b2_e)
    NCST = N_EXP * MC + N_ROUTED * MC + OC
    cst_d = nc.declare_dram_parameter("cst", [128, NCST], F32, isOutput=False)
    outT_d = nc.declare_dram_parameter("outT", [OUT_F, nl], F32, isOutput=True)

    with tile.TileContext(nc) as tc:
        with (
            tc.tile_pool(name="consts", bufs=1) as consts,
            tc.tile_pool(name="x8p", bufs=4) as x8p,
            tc.tile_pool(name="xbp", bufs=4) as xbp,
            tc.tile_pool(name="w6p", bufs=2) as w6p,
            tc.tile_pool(name="wbcp", bufs=4) as wbcp,
            tc.tile_pool(name="hps", bufs=4, space="PSUM") as hpsp,
            tc.tile_pool(name="hp", bufs=10) as hp_pool,
            tc.tile_pool(name="hpm", bufs=3) as hpm_pool,
            tc.tile_pool(name="ops", bufs=4, space="PSUM") as outps_pool,
            tc.tile_pool(name="osb", bufs=3) as osb_pool,
        ):
            # ---- constants / weights preload ----
            cst_sb = consts.tile([128, NCST], F32, name="cst_sb")
            AOFF = N_EXP * MC          # alpha column offset in cst
            B2OFF = AOFF + N_ROUTED * MC  # b2s column offset in cst
            ones_sb = consts.tile([1, TILE_N], BF16, name="ones_sb")

            # Per-expert weight tiles, streamed in first-use order so tile 0
            # can start after the first experts' weights land.
            eorder = []
            for _, routed in tiles:
                for e in list(routed) + [N_ROUTED + s for s in range(N_SHARED)]:
                    if e not in eorder:
                        eorder.append(e)
                if len(eorder) == N_EXP:
                    break
            for e in range(N_EXP):
                if e not in eorder:
                    eorder.append(e)

            w1t_v = w1t_d.rearrange("e (k p) h -> e p k h", p=128)
            w2t_v = w2t_d.rearrange("e (m p) o -> e p m o", p=128)
            w1s_v = w1s_d.rearrange("e (k p) h -> e p k h", p=128)
            w2s_v = w2s_d.rearrange("e (m p) o -> e p m o", p=128)
            w1_sb = {}
            w2_sb = {}

            def load_w1(e):
                if e < N_ROUTED:
                    w1_sb[e] = consts.tile([128, KC, HID], FP8, name=f"w1e{e}")
                    nc.sync.dma_start(w1_sb[e][:], w1t_v[e])
                else:
                    w1_sb[e] = consts.tile([128, KC, HID], BF16, name=f"w1e{e}")
                    nc.sync.dma_start(w1_sb[e][:], w1s_v[e - N_ROUTED])

            def load_w2(e):
                if e < N_ROUTED:
                    w2_sb[e] = consts.tile([128, MC, OUT_F], FP8, name=f"w2e{e}")
                    nc.sync.dma_start(w2_sb[e][:], w2t_v[e])
                else:
                    w2_sb[e] = consts.tile([128, MC, OUT_F], BF16, name=f"w2e{e}")
                    nc.sync.dma_start(w2_sb[e][:], w2s_v[e - N_ROUTED])

            def load_expert_weights(e):
                load_w1(e)
                load_w2(e)

            # DMA issue order is the critical path at kernel start (~700ns
            # queue cost per issue): tile 0's x8 + first expert's W1 + silu
            # consts lead, the rest streams behind in first-use order.
            x8_v = x8_d.rearrange("(k p) a -> p k a", p=128)
            xb_v = xb_d.rearrange("(k p) a -> p k a", p=128)
            n0 = tiles[0][0]
            t0_x8 = x8p.tile([128, KC, TILE_N], FP8, name="x8_sb", tag="x8")
            nc.sync.dma_start(t0_x8[:, :, :n0], x8_v[:, :, 0:n0])
            load_w1(eorder[0])
            nc.sync.dma_start(cst_sb[:], cst_d[:])
            t0_w6 = w6p.tile([1, N_ROUTED, TILE_N], BF16, name="w6row", tag="w6r")
            nc.sync.dma_start(t0_w6[0:1, :, :n0], w6_d[:, 0:n0])
            load_w1(eorder[1])
            t0_xb = xbp.tile([128, KC, TILE_N], BF16, name="xb_sb", tag="xb")
            nc.sync.dma_start(t0_xb[:, :, :n0], xb_v[:, :, 0:n0])
            load_w1(eorder[2])
            load_w2(eorder[0])
            load_w2(eorder[1])

            # Warm the PE HAM clock gate (cold = 1.2 instead of 2.4 GHz) and
            # the ScalarE activation table while the first DMAs are in flight.
            nc.vector.memset(ones_sb[:], 1.0)
            warm_sb = consts.tile([128, 1], F32, name="warm_sb")
            for _ in range(8):
                warm_ps = hpsp.tile([128, TILE_N], F32, name="warm_ps", tag="hps")
                nc.tensor.matmul(
                    warm_ps[:, :], ones_sb[0:1, 0:128], ones_sb[0:1, :],
                    start=True, stop=True,
                )
            # short filler matmuls bridge the dep-wait between the warm spin
            # and tile 0's first real matmul in ~100ns steps, so the PE's
            # LDWEIGHTS-ahead pipeline stays primed when the real stream
            # starts (first real MMs otherwise pay serial LDW, ~420 vs 212ns)
            for _ in range(8):
                warm_ps = hpsp.tile([128, TILE_N], F32, name="warm_ps", tag="hps")
                nc.tensor.matmul(
                    warm_ps[:, 0:64], ones_sb[0:1, 0:128], ones_sb[0:1, 0:64],
                    start=True, stop=True,
                )
            nc.scalar.activation(
                warm_sb[:, :], cst_sb[:, 0:1],
                mybir.ActivationFunctionType.Silu,
            )

            # ---- main loop over atom tiles ----
            outT_v = outT_d.rearrange("(c p) a -> p c a", p=128)
            a0 = 0
            for t, (n, routed) in enumerate(tiles):
                experts = list(routed) + [N_ROUTED + s for s in range(N_SHARED)]

                if t == 0:
                    x8_sb, xb_sb, w6row = t0_x8, t0_xb, t0_w6
                    # tile 0 still needs the shared experts' remaining
                    # weights; the cold experts (first used at tile ~7+)
                    # spread over tiles 1..4 below so their ~700ns/issue
                    # queue cost doesn't delay tiles 1-4's input prefetch
                    load_w2(eorder[2])
                    load_expert_weights(eorder[3])
                else:
                    x8_sb = x8p.tile([128, KC, TILE_N], FP8, name="x8_sb", tag="x8")
                    nc.sync.dma_start(x8_sb[:, :, :n], x8_v[:, :, a0 : a0 + n])
                    xb_sb = xbp.tile([128, KC, TILE_N], BF16, name="xb_sb", tag="xb")
                    nc.sync.dma_start(xb_sb[:, :, :n], xb_v[:, :, a0 : a0 + n])
                    # gate rows packed onto partition 0
                    w6row = w6p.tile([1, N_ROUTED, TILE_N], BF16, name="w6row", tag="w6r")
                    nc.sync.dma_start(w6row[0:1, :, :n], w6_d[:, a0 : a0 + n])
                    if t - 1 < len(eorder[4:]):
                        load_expert_weights(eorder[4 + t - 1])

                # per-atom gates broadcast across 128 partitions (GPSIMD,
                # keeps PE free)
                wsb = {}
                for e in routed:
                    wsb_e = wbcp.tile([128, TILE_N], BF16, name="wsb", tag="wbc")
                    nc.gpsimd.partition_broadcast(
                        wsb_e[:, :n], w6row[0:1, e, :n]
                    )
                    wsb[e] = wsb_e

                # output accumulators (opened by the first mm2 below)
                outps = [
                    outps_pool.tile([128, TILE_N], F32, name="ops", tag="ops")
                    for _ in range(OC)
                ]
                state = {"first": True}

                def emit_mm2(e, shared, src, last_e):
                    # src: hpm tile (routed) or list of h tiles (shared)
                    if shared:
                        for m in range(MC):
                            for c in range(OC):
                                nc.tensor.matmul(
                                    outps[c][:, :n],
                                    w2_sb[e][:, m, c * 128 : (c + 1) * 128],
                                    src[m][:, :n],
                                    start=state["first"],
                                    stop=(last_e and m == MC - 1),
                                )
                            state["first"] = False
                    else:
                        for mp in range(MC // 2):
                            for c in range(OC):
                                nc.tensor.matmul(
                                    outps[c][:, :n],
                                    w2_sb[e][:, 2 * mp : 2 * mp + 2, c * 128 : (c + 1) * 128],
                                    src[:, 2 * mp : 2 * mp + 2, :n],
                                    start=state["first"],
                                    stop=(last_e and mp == MC // 2 - 1),
                                    perf_mode=DR,
                                )
                            state["first"] = False

                # software pipeline: expert i's mm2 block is emitted after
                # expert i+1's mm1 block, so the silu/STT chain feeding each
                # mm2 is covered by PE work instead of stalling the PE
                pending = None
                for ei, e in enumerate(experts):
                    last_e = ei == len(experts) - 1
                    shared = e >= N_ROUTED
                    if shared:
                        src = []
                    else:
                        src = hpm_pool.tile(
                            [128, MC, TILE_N], FP8, name="hpm_sb", tag="hpm"
                        )
                    for m in range(MC):
                        hps = hpsp.tile([128, TILE_N], F32, name="hps", tag="hps")
                        if shared:
                            for k in range(KC):
                                nc.tensor.matmul(
                                    hps[:, :n],
                                    w1_sb[e][:, k, m * 128 : (m + 1) * 128],
                                    xb_sb[:, k, :n],
                                    start=(k == 0),
                                    stop=(k == KC - 1),
                                )
                        else:
                            for kp in range(KC // 2):
                                nc.tensor.matmul(
                                    hps[:, :n],
                                    w1_sb[e][:, 2 * kp : 2 * kp + 2, m * 128 : (m + 1) * 128],
                                    x8_sb[:, 2 * kp : 2 * kp + 2, :n],
                                    start=(kp == 0),
                                    stop=(kp == KC // 2 - 1),
                                    perf_mode=DR,
                                )
                        h_sb = hp_pool.tile([128, TILE_N], BF16, name="h_sb", tag="h")
                        nc.scalar.activation(
                            h_sb[:, :n], hps[:, :n],
                            mybir.ActivationFunctionType.Silu,
                            bias=cst_sb[:, e * MC + m : e * MC + m + 1],
                            scale=(1.0 if shared else 1.0 / S1),
                        )
                        if shared:
                            src.append(h_sb)
                        else:
                            ac = e * MC + m
                            nc.vector.scalar_tensor_tensor(
                                src[:, m, :n],
                                h_sb[:, :n],
                                cst_sb[:, AOFF + ac : AOFF + ac + 1],
                                wsb[e][:, :n],
                                mybir.AluOpType.add,
                                mybir.AluOpType.mult,
                            )
                    if pending is not None:
                        emit_mm2(*pending)
                    pending = (e, shared, src, last_e)
                emit_mm2(*pending)

                # psum -> sbuf: uniform 1/S2R descale + shared b2 bias (DVE)
                osb = osb_pool.tile([128, OC, TILE_N], F32, name="osb", tag="osb")
                for c in range(OC):
                    nc.vector.tensor_scalar(
                        osb[:, c, :n], outps[c][:, :n],
                        1.0 / S2R, cst_sb[:, B2OFF + c : B2OFF + c + 1],
                        mybir.AluOpType.mult, mybir.AluOpType.add,
                    )
                nc.sync.dma_start(
                    outT_v[:, :, a0 : a0 + n], osb[:, :, :n]
                )
                a0 += n

    nc.compile()
    return nc


def _alpha_pack(rW2, rb2):
    """alpha_e = min-norm solution of W2_e @ alpha = b2_e, packed per-chunk."""
    alphas = []
    for e in range(N_ROUTED):
        a, *_ = np.linalg.lstsq(rW2[e].astype(np.float64), rb2[e].astype(np.float64))
        alphas.append(a)
    al = np.stack(alphas).astype(np.float32)  # [6, HID]
    return np.ascontiguousarray(
        al.reshape(N_ROUTED, MC, 128).transpose(2, 0, 1).reshape(128, N_ROUTED * MC)
    )


def _prep_host(inputs):
    feats = np.asarray(inputs["features"], dtype=np.float32)
    species = np.asarray(inputs["species_idx"]).astype(np.int64)
    emb = np.asarray(inputs["emb"], dtype=np.float32)
    Wr = np.asarray(inputs["W_router"], dtype=np.float32)
    rW1 = np.asarray(inputs["rW1"], dtype=np.float32)
    rb1 = np.asarray(inputs["rb1"], dtype=np.float32)
    rW2 = np.asarray(inputs["rW2"], dtype=np.float32)
    rb2 = np.asarray(inputs["rb2"], dtype=np.float32)
    sW1 = np.asarray(inputs["sW1"], dtype=np.float32)
    sb1 = np.asarray(inputs["sb1"], dtype=np.float32)
    sW2 = np.asarray(inputs["sW2"], dtype=np.float32)
    sb2 = np.asarray(inputs["sb2"], dtype=np.float32)

    wt_table, top2 = _router_table(emb, Wr)
    idx_cores, tiles = _plan_sharding(species, top2)
    nl = idx_cores.shape[1]
    w_atoms = wt_table[species] * SH  # [n, 6] f32, pre-scaled gates

    b1 = np.concatenate([rb1, sb1], axis=0)  # [8, HID]

    shared = {
        "w1t": np.ascontiguousarray(
            (rW1 * S1).transpose(0, 2, 1)
        ).astype(FP8_NP),
        "w2t": np.ascontiguousarray(
            (rW2 * (S2R / SH)).transpose(0, 2, 1)
        ).astype(FP8_NP),
        "w1s": np.ascontiguousarray(sW1.transpose(0, 2, 1)).astype(BF16_NP),
        "w2s": np.ascontiguousarray(
            (sW2 * S2R).transpose(0, 2, 1)
        ).astype(BF16_NP),
        "cst": np.ascontiguousarray(
            np.concatenate(
                [
                    b1.reshape(N_EXP, MC, 128)
                    .transpose(2, 0, 1)
                    .reshape(128, N_EXP * MC),
                    _alpha_pack(rW2, rb2),
                    sb2.sum(axis=0).reshape(OC, 128).T,
                ],
                axis=1,
            ).astype(np.float32)
        ),
    }

    in_maps = []
    for c in range(N_CORES):
        idx = idx_cores[c]
        valid = idx >= 0
        iv = idx[valid]
        fT = np.ascontiguousarray(feats[iv].T)
        x8 = np.zeros((IN_F, nl), dtype=FP8_NP)
        x8[:, valid] = fT.astype(FP8_NP)
        xb = np.zeros((IN_F, nl), dtype=BF16_NP)
        xb[:, valid] = fT.astype(BF16_NP)
        w6 = np.zeros((N_ROUTED, nl), dtype=BF16_NP)
        w6[:, valid] = np.ascontiguousarray(w_atoms[iv].T).astype(BF16_NP)
        in_maps.append({"x8": x8, "xb": xb, "w6": w6, **shared})
    return in_maps, idx_cores, tiles, nl, feats.shape[0]


_PROGRAM_CACHE = {}


def _get_program(nl, tiles):
    key = (nl, tuple(tiles))
    if key not in _PROGRAM_CACHE:
        _PROGRAM_CACHE[key] = _build_program(nl, tiles)
    return _PROGRAM_CACHE[key]


# Set TRACE=True (e.g. from a test harness) to capture a neuron-profile trace;
# the full BassKernelResults of the last run is kept in LAST_RESULTS.
TRACE = False
LAST_RESULTS = None


def kernel(**inputs):
    global LAST_RESULTS
    in_maps, idx_cores, tiles, nl, n_atoms = _prep_host(inputs)
    nc = _get_program(nl, tiles)
    res = run_bass_kernel_spmd(nc, in_maps, list(range(N_CORES)), trace=TRACE)
    LAST_RESULTS = res
    out = np.zeros((n_atoms, OUT_F), dtype=np.float32)
    for c in range(N_CORES):
        idx = idx_cores[c]
        valid = idx >= 0
        outT = res.results[c]["outT"]  # [OUT_F, nl] f32
        out[idx[valid]] = outT[:, valid].T
    return out
